# revision 12
# baseline (speedup 1.0000x reference)
"""Discretized-mixture NLL, Trainium2, v2: transposed layout (pixels on partitions).

Per core (data parallel over batch): n_px = nb*4096 pixels, processed as 128
chunks of 128 px. Conv = per-chunk matmul with lhsT = pz-chunk [128ch, 128px]
(stationary, bf16, FWL) x rhs = W [128ch, 96params] (moving), accumulated over
4 K-chunks into psum [128px, 16*128] slabs (16 chunks per group, cols 128j+0:96).

Param col order (within 96): [pi(30) | mu(30) | ls(30) | pad(6)], intra-block
c*10+m, so mixture reduction is a stride-10 innermost free-dim reduce_sum.

All elementwise ops run full 128-partition width:
  D = mu - xe (DVE, xe broadcast via stride-0 AP); tanh(ls/8) (ACT);
  g = exp(pi), s2 = exp(-8t + ln(1/sqrt2)) (ACT); hi/lo = (D -+ d)*s2 (DVE STT);
  erf [128,960] (ACT); dlt = elo-ehi, qd = g*dlt (GpSimd); num/den = reduce_sum
  (DVE); tail: num2 = num + 2eps*den, nll = ln(2*den) - ln(num2) (log-2 folding
  absorbs the 0.5 in dcdf), one [128, 512] f32 output DMA, host untangles.

ACT ops are chained in table-set phases (exp-set / erf-set per 4-group
supergroup, ln at the end) to bound ACT_TABLE_LOADs at ~5.
"""

import numpy as np

WIDTH = 512
C_IMG = 3
N_MIX = 10
SIZE = 64
STD = 127.5
EPS = 1e-8
DELTA = 1.0 / STD / 2.0
LOG_INV_SQRT2 = -0.34657359027997264
N_CORES = 8
CHUNK = 128            # pixels per matmul chunk (psum partitions)
G_CH = 16              # chunks per group (one psum slab)
SG = 4                 # groups per ACT table-set supergroup


def make_consts_v2(W, b):
    import ml_dtypes

    W = np.asarray(W, np.float32)
    b = np.asarray(b, np.float32)
    Wp = np.zeros((96, WIDTH), np.float32)
    bp = np.zeros(96, np.float32)
    for j in range(30):
        c, m = j // 10, j % 10
        Wp[j] = W[m * 3 + c]                  # pi
        Wp[30 + j] = W[(10 + m) * 3 + c]      # mu
        Wp[60 + j] = W[(20 + m) * 3 + c]      # logsigma
        bp[j] = b[m * 3 + c]
        bp[30 + j] = b[(10 + m) * 3 + c]
        bp[60 + j] = b[(20 + m) * 3 + c]
    wt = np.ascontiguousarray(Wp.T.astype(ml_dtypes.bfloat16))   # [512, 96]
    return wt, bp


def build_nc_v2(n_batch=4, with_bias=False):
    from contextlib import ExitStack

    import concourse.bacc as bacc
    import concourse.mybir as mybir
    import concourse.tile as tile
    from concourse.bass import broadcast_tensor_aps
    from concourse.tile import add_dep_helper

    f32 = mybir.dt.float32
    f32r = mybir.dt.float32r
    bf16 = mybir.dt.bfloat16
    ALU = mybir.AluOpType
    ACT = mybir.ActivationFunctionType
    AXL = mybir.AxisListType

    n_px = n_batch * SIZE * SIZE
    n_ch = n_px // CHUNK                       # 128 chunks
    n_g = n_ch // G_CH                         # 8 groups
    px_g = G_CH * CHUNK                        # 2048 px per group
    assert n_g % SG == 0

    nc = bacc.Bacc("TRN2", target_bir_lowering=False, debug=False)
    pz = nc.dram_tensor("pz", [n_batch, WIDTH, SIZE * SIZE], bf16,
                        kind="ExternalInput").ap()
    xp = nc.dram_tensor("xp", [128, n_ch * 3], f32, kind="ExternalInput").ap()
    wt = nc.dram_tensor("wt", [WIDTH, 96], bf16, kind="ExternalInput").ap()
    bv = nc.dram_tensor("bv", [1, 96], f32, kind="ExternalInput").ap()
    out = nc.dram_tensor("out", [128, n_ch * 4], f32, kind="ExternalOutput").ap()

    with tile.TileContext(nc) as tc, ExitStack() as ctx:
        const_pool = ctx.enter_context(tc.tile_pool(name="const", bufs=1))
        xt_pool = ctx.enter_context(tc.tile_pool(name="xt", bufs=4))
        d_pool = ctx.enter_context(tc.tile_pool(name="d", bufs=2))
        t_pool = ctx.enter_context(tc.tile_pool(name="t", bufs=2))
        s2_pool = ctx.enter_context(tc.tile_pool(name="s2", bufs=2))
        g_pool = ctx.enter_context(tc.tile_pool(name="g", bufs=SG + 1))
        hl_pool = ctx.enter_context(tc.tile_pool(name="hl", bufs=SG + 1))
        e_pool = ctx.enter_context(tc.tile_pool(name="e", bufs=2))
        dq_pool = ctx.enter_context(tc.tile_pool(name="dq", bufs=4))
        tail_pool = ctx.enter_context(tc.tile_pool(name="tail", bufs=1))
        ps_pool = ctx.enter_context(tc.tile_pool(name="ps", bufs=2, space="PSUM"))

        wt_sb = const_pool.tile([128, 4 * 96], bf16)
        nc.sync.dma_start(
            wt_sb[:].rearrange("i (k o) -> i k o", o=96),
            wt.rearrange("(k i) o -> i k o", i=128),
        )
        xp_sb = const_pool.tile([128, n_ch * 3], f32)
        nc.sync.dma_start(xp_sb[:], xp)
        if with_bias:
            bv_sb = const_pool.tile([1, 96], f32)
            nc.sync.dma_start(bv_sb[:], bv)
            ones_sb = const_pool.tile([1, 128], f32)
            nc.vector.memset(ones_sb[:], 1.0)

        num_slab = tail_pool.tile([128, n_ch * 4], f32, name="num_slab")
        den_slab = tail_pool.tile([128, n_ch * 4], f32, name="den_slab")
        nc.vector.memset(num_slab[:], 1.0)
        nc.vector.memset(den_slab[:], 1.0)
        cb_sb = const_pool.tile([128, 1], f32)
        nc.vector.memset(cb_sb[:], LOG_INV_SQRT2)

        act_chain = []

        def chain(inst):
            if act_chain:
                add_dep_helper(inst.ins, act_chain[-1].ins, sync=False,
                               reason="act table-set batching")
            act_chain.append(inst)
            return inst

        def phase_a(g):
            """DMA + conv matmuls + D/tanh/exp/STT for one 2048-px group."""
            b, half = divmod(g, (SIZE * SIZE) // px_g * 1)
            b = g // ((SIZE * SIZE) // px_g)
            half = g % ((SIZE * SIZE) // px_g)
            off = half * px_g
            xt_t = xt_pool.tile([128, 4 * px_g], bf16, tag="xt")
            xt_v = xt_t[:].rearrange("i (k n) -> i k n", n=px_g)
            pz_v = pz[b, :, off:off + px_g].rearrange("(k i) n -> i k n", i=128)
            nc.sync.dma_start(xt_v[:, 0:2], pz_v[:, 0:2])
            nc.scalar.dma_start(xt_v[:, 2:4], pz_v[:, 2:4])

            ps = ps_pool.tile([128, G_CH * 128], f32, tag="ps")
            for j in range(G_CH):
                for k in range(4):
                    nc.tensor.matmul(
                        ps[:, 128 * j:128 * j + 96],
                        xt_v[:, k, 128 * j:128 * (j + 1)],
                        wt_sb[:, 96 * k:96 * (k + 1)],
                        start=(k == 0), stop=(k == 3 and not with_bias),
                    )
                if with_bias:
                    nc.tensor.matmul(
                        ps[:, 128 * j:128 * j + 96],
                        ones_sb[:].bitcast(f32r), bv_sb[:].bitcast(f32r),
                        start=False, stop=True,
                    )
            ps_v = ps[:].rearrange("p (j x) -> p j x", x=128)
            pi_ap = ps_v[:, :, 0:30]
            mu_ap = ps_v[:, :, 30:60].rearrange("p j (c m) -> p j c m", m=10)
            ls_ap = ps_v[:, :, 60:90]

            # D = mu - xe  (xe broadcast over mixtures via stride-0 AP)
            d_t = d_pool.tile([128, G_CH * 30], f32, tag="d")
            d_v = d_t[:].rearrange("p (j c m) -> p j c m", c=3, m=10)
            xe_v = xp_sb[:, 3 * G_CH * g: 3 * G_CH * (g + 1)].rearrange(
                "p (j c m) -> p j c m", c=3, m=1)
            mu_b, xe_b = broadcast_tensor_aps(mu_ap, xe_v)
            nc.vector.tensor_tensor(d_v, mu_b, xe_b, ALU.subtract)

            t_t = t_pool.tile([128, G_CH * 30], f32, tag="t")
            t_v = t_t[:].rearrange("p (j y) -> p j y", y=30)
            chain(nc.scalar.activation(t_v, ls_ap, ACT.Tanh, scale=0.125))
            g_t = g_pool.tile([128, G_CH * 30], f32, tag="g")
            g_v = g_t[:].rearrange("p (j y) -> p j y", y=30)
            chain(nc.scalar.activation(g_v, pi_ap, ACT.Exp))
            s2_t = s2_pool.tile([128, G_CH * 30], f32, tag="s2")
            chain(nc.scalar.activation(s2_t[:], t_t[:], ACT.Exp,
                                       bias=cb_sb[:], scale=-8.0))

            hl_t = hl_pool.tile([128, G_CH * 60], f32, tag="hl")
            nw = G_CH * 30
            nc.vector.scalar_tensor_tensor(
                hl_t[:, 0:nw], d_t[:], DELTA, s2_t[:], ALU.subtract, ALU.mult)
            nc.vector.scalar_tensor_tensor(
                hl_t[:, nw:2 * nw], d_t[:], DELTA, s2_t[:], ALU.add, ALU.mult)
            # den depends only on g -> reduce here so it overlaps the stream
            den_v = den_slab[:].rearrange("p (ch q) -> p ch q", q=4)
            nc.vector.reduce_sum(
                den_v[:, G_CH * g:G_CH * (g + 1), 0:3],
                g_t[:].rearrange("p (j c m) -> p j c m", c=3, m=10),
                axis=AXL.X)
            return g_t, hl_t

        def phase_b(g, g_t, hl_t):
            """erf + mixture reduction for one group."""
            nw = G_CH * 30
            e_t = e_pool.tile([128, G_CH * 60], f32, tag="e")
            chain(nc.scalar.activation(e_t[:], hl_t[:], ACT.Erf))
            dlt_t = dq_pool.tile([128, nw], f32, tag="dlt")
            nc.vector.tensor_tensor(dlt_t[:], e_t[:, nw:2 * nw], e_t[:, 0:nw],
                                    ALU.subtract)
            qd_t = dq_pool.tile([128, nw], f32, tag="qd")
            nc.vector.tensor_tensor(qd_t[:], g_t[:], dlt_t[:], ALU.mult)
            num_v = num_slab[:].rearrange("p (ch q) -> p ch q", q=4)
            nc.vector.reduce_sum(
                num_v[:, G_CH * g:G_CH * (g + 1), 0:3],
                qd_t[:].rearrange("p (j c m) -> p j c m", c=3, m=10),
                axis=AXL.X)

        # Interleave A (exp-set) and B (erf-set) brackets so B work overlaps
        # the DMA-paced A stream instead of piling up in a serial tail:
        # [A0 A1][B0][A2 A3][B1 B2][A4 A5][B3 B4][A6 A7][B5 B6 B7]
        num2 = tail_pool.tile([128, n_ch * 4], f32, name="num2")
        ln_n = tail_pool.tile([128, n_ch * 4], f32, name="ln_n")
        ln_d = tail_pool.tile([128, n_ch * 4], f32, name="ln_d")
        nll = tail_pool.tile([128, n_ch * 4], f32, name="nll")
        nh = n_ch * 4 // 2

        def do_tail(half):
            sl = slice(nh * half, nh * (half + 1))
            nc.vector.scalar_tensor_tensor(num2[:, sl], den_slab[:, sl],
                                           2.0 * EPS, num_slab[:, sl],
                                           ALU.mult, ALU.add)
            chain(nc.scalar.activation(ln_n[:, sl], num2[:, sl], ACT.Ln))
            chain(nc.scalar.activation(ln_d[:, sl], den_slab[:, sl], ACT.Ln,
                                       scale=2.0))
            nc.vector.tensor_tensor(nll[:, sl], ln_d[:, sl], ln_n[:, sl],
                                    ALU.subtract)
            nc.sync.dma_start(out[:, sl], nll[:, sl])

        ab = {}
        done_b = 0
        for pair in range(n_g // 2):
            for g in (2 * pair, 2 * pair + 1):
                ab[g] = phase_a(g)
            if pair == 0:
                continue
            hi = n_g - 1 if pair == n_g // 2 - 1 else 2 * pair + 1
            while done_b <= hi:
                phase_b(done_b, *ab.pop(done_b))
                done_b += 1
            if done_b == n_g // 2 + 2:
                do_tail(0)      # groups 0..3 complete: first-half ln + output
        do_tail(1)

    nc.compile()
    return nc


def prep_core_inputs_v2(px_z_shard, x_shard, consts):
    import ml_dtypes

    wt, bp = consts
    nb = px_z_shard.shape[0]
    n_px = nb * SIZE * SIZE
    n_ch = n_px // CHUNK
    pzs = np.ascontiguousarray(
        px_z_shard.reshape(nb, WIDTH, SIZE * SIZE).astype(ml_dtypes.bfloat16))
    xc = np.asarray(x_shard, np.float32).reshape(n_px, C_IMG)
    xpp = np.ascontiguousarray(
        xc.reshape(n_ch, CHUNK, C_IMG).transpose(1, 0, 2).reshape(128, n_ch * 3))
    return {"pz": pzs, "xp": xpp, "wt": wt,
            "bv": np.ascontiguousarray(bp[None, :])}


def gather_core_output_v2(o, nb):
    n_px = nb * SIZE * SIZE
    n_ch = n_px // CHUNK
    o = o.reshape(128, n_ch, 4)[:, :, 0:3]          # [p, ch, c]
    return (o.transpose(1, 0, 2).reshape(nb, SIZE, SIZE, C_IMG))


_NC_CACHE = {}


def kernel(px_z, x, W, b):
    from concourse.bass_utils import run_bass_kernel_spmd

    px_z = np.asarray(px_z, np.float32)
    x = np.asarray(x, np.float32)
    B = px_z.shape[0]
    nb = B // N_CORES
    consts = make_consts_v2(W, b)
    with_bias = bool(np.any(np.asarray(b) != 0))
    key = (nb, with_bias)
    if key not in _NC_CACHE:
        _NC_CACHE[key] = build_nc_v2(n_batch=nb, with_bias=with_bias)
    nc = _NC_CACHE[key]
    in_maps = [
        prep_core_inputs_v2(px_z[nb * i:nb * (i + 1)], x[nb * i:nb * (i + 1)],
                            consts)
        for i in range(N_CORES)
    ]
    res = run_bass_kernel_spmd(nc, in_maps, core_ids=list(range(N_CORES)))
    outs = [gather_core_output_v2(res.results[i]["out"], nb)
            for i in range(N_CORES)]
    return np.concatenate(outs, 0)
